# revision 3
# baseline (speedup 1.0000x reference)
"""Trainium2 Bass kernel for nn_LocalEncoder (masked GRU + attention pooling).

Strategy (data-parallel over batch, 8 cores x 512 rows, 2 chunks of 256/core):
- Feature-major layout [U partitions, batch free]. All matmuls bf16 -> fp32 PSUM.
- Scan: per timestep, 6 matmuls (x-proj + recurrent) with biases and the
  trailing-padding mask folded in via augmented x rows (mask row scaled by -40
  makes the update gate ~0 on masked steps, freezing h exactly like the
  reference's jnp.where).
- all_state spilled to DRAM (bf16), re-streamed for the attention phase.
- Attention computed with the last-state term UNMASKED on device; the host
  subtracts the closed-form correction for masked (trailing) timesteps:
  masked t contribute (T-len)*sigmoid(last@A2)@v * last, device counted
  (T-len)*sigmoid(last@A2 + last@A1)@v * last.
"""
import sys
sys.path.insert(0, "/opt/trn_rl_repo")
from contextlib import ExitStack

import numpy as np
import ml_dtypes

import concourse.bass as bass
import concourse.bacc as bacc
import concourse.tile as tile
from concourse import mybir
from concourse import bass_utils

bf16 = ml_dtypes.bfloat16
AF = mybir.ActivationFunctionType
OP = mybir.AluOpType

B, T, E, U = 4096, 200, 100, 100
NCORES = 8
BC = 256          # chunk width (free dim of every op)
NCHUNK = 2        # chunks per core; BC*NCHUNK = per-core batch
PERCORE = BC * NCHUNK

_CACHE = {}


def _build():
    nc = bacc.Bacc()
    dt = mybir.dt
    xaug = nc.dram_tensor("xaug", [T, NCHUNK, 128, BC], dt.bfloat16, kind="ExternalInput")
    wKzN = nc.dram_tensor("wKzN", [128, U], dt.bfloat16, kind="ExternalInput")
    wKr = nc.dram_tensor("wKr", [128, U], dt.bfloat16, kind="ExternalInput")
    wKh = nc.dram_tensor("wKh", [128, U], dt.bfloat16, kind="ExternalInput")
    wRzN = nc.dram_tensor("wRzN", [U, U], dt.bfloat16, kind="ExternalInput")
    wRr = nc.dram_tensor("wRr", [U, U], dt.bfloat16, kind="ExternalInput")
    wRh = nc.dram_tensor("wRh", [U, U], dt.bfloat16, kind="ExternalInput")
    wb1h = nc.dram_tensor("wb1h", [1, U], dt.bfloat16, kind="ExternalInput")
    wA1 = nc.dram_tensor("wA1", [U, U], dt.bfloat16, kind="ExternalInput")
    wA2 = nc.dram_tensor("wA2", [U, U], dt.bfloat16, kind="ExternalInput")
    wVr = nc.dram_tensor("wVr", [U, U], dt.bfloat16, kind="ExternalInput")
    wI = nc.dram_tensor("wI", [U, U], dt.bfloat16, kind="ExternalInput")
    outraw = nc.dram_tensor("outraw", [NCHUNK, U, BC], dt.float32, kind="ExternalOutput")
    lastout = nc.dram_tensor("lastout", [NCHUNK, U, BC], dt.float32, kind="ExternalOutput")

    with tile.TileContext(nc) as tc, ExitStack() as octx:
        singles = octx.enter_context(tc.tile_pool(name="singles", bufs=1))
        dram = octx.enter_context(tc.tile_pool(name="dram", bufs=1, space="DRAM"))

        # persistent weights
        def load_w(dram_w, p):
            t = singles.tile([p, U], mybir.dt.bfloat16, tag=dram_w.name)
            nc.sync.dma_start(out=t, in_=dram_w[:, :])
            return t
        KzN, Kr, Kh = load_w(wKzN, 128), load_w(wKr, 128), load_w(wKh, 128)
        RzN, Rr, Rh = load_w(wRzN, U), load_w(wRr, U), load_w(wRh, U)
        A1b, A2b, Vr, I100 = load_w(wA1, U), load_w(wA2, U), load_w(wVr, U), load_w(wI, U)
        b1h = singles.tile([1, U], mybir.dt.bfloat16, tag="b1h")
        nc.sync.dma_start(out=b1h, in_=wb1h[:, :])
        ones = singles.tile([1, BC], mybir.dt.bfloat16, tag="ones")
        nc.vector.memset(ones, 1.0)

        state = dram.tile([NCHUNK, T, U, BC], mybir.dt.bfloat16)
        last_tiles = []

        # ---------------- scan ----------------
        with ExitStack() as ctx:
            xp = ctx.enter_context(tc.tile_pool(name="xp", bufs=4))
            hp = ctx.enter_context(tc.tile_pool(name="hp", bufs=3))
            gp = ctx.enter_context(tc.tile_pool(name="gp", bufs=3))
            pzr = ctx.enter_context(tc.tile_pool(name="pzr", bufs=1, space="PSUM"))
            pxh = ctx.enter_context(tc.tile_pool(name="pxh", bufs=1, space="PSUM"))
            prh = ctx.enter_context(tc.tile_pool(name="prh", bufs=1, space="PSUM"))

            hprev = [None] * NCHUNK
            for c in range(NCHUNK):
                h0 = hp.tile([128, BC], mybir.dt.bfloat16, tag=f"h{c}")
                nc.vector.memset(h0, 0.0)
                hprev[c] = h0

            for t in range(T):
                for c in range(NCHUNK):
                    xt = xp.tile([128, BC], mybir.dt.bfloat16, tag=f"x{c}")
                    nc.sync.dma_start(out=xt, in_=xaug[t, c, :, :])
                    h = hprev[c]
                    zr = pzr.tile([128, 2, 512], mybir.dt.float32, tag=f"zr{c}")
                    xh = pxh.tile([128, 512], mybir.dt.float32, tag=f"xh{c}")
                    rh = prh.tile([128, 512], mybir.dt.float32, tag=f"rh{c}")
                    nc.tensor.matmul(zr[0:U, 0, 0:BC], lhsT=KzN, rhs=xt, start=True, stop=False)
                    nc.tensor.matmul(zr[0:U, 0, 0:BC], lhsT=RzN, rhs=h[0:U, :], start=False, stop=True)
                    nc.tensor.matmul(zr[0:U, 1, 0:BC], lhsT=Kr, rhs=xt, start=True, stop=False)
                    nc.tensor.matmul(zr[0:U, 1, 0:BC], lhsT=Rr, rhs=h[0:U, :], start=False, stop=True)
                    nc.tensor.matmul(xh[0:U, 0:BC], lhsT=Kh, rhs=xt, start=True, stop=True)
                    nc.tensor.matmul(rh[0:U, 0:BC], lhsT=Rh, rhs=h[0:U, :], start=True, stop=False)
                    nc.tensor.matmul(rh[0:U, 0:BC], lhsT=b1h, rhs=ones, start=False, stop=True)
                    # gates: one sigmoid over both banks (zcm | r)
                    zrs = gp.tile([U, 2, BC], mybir.dt.bfloat16, tag=f"zrs{c}")
                    nc.scalar.activation(zrs[:, :, :], zr[0:U, :, 0:BC], AF.Sigmoid)
                    t1 = gp.tile([U, BC], mybir.dt.bfloat16, tag=f"t1{c}")
                    nc.vector.tensor_tensor(t1, zrs[:, 1, :], rh[0:U, 0:BC], OP.mult)
                    s = gp.tile([U, BC], mybir.dt.bfloat16, tag=f"s{c}")
                    nc.vector.tensor_tensor(s, xh[0:U, 0:BC], t1, OP.add)
                    hh = gp.tile([U, BC], mybir.dt.bfloat16, tag=f"hh{c}")
                    nc.scalar.activation(hh, s, AF.Tanh)
                    d = gp.tile([U, BC], mybir.dt.bfloat16, tag=f"d{c}")
                    nc.vector.tensor_tensor(d, hh, h[0:U, :], OP.subtract)
                    e = gp.tile([U, BC], mybir.dt.bfloat16, tag=f"e{c}")
                    nc.vector.tensor_tensor(e, zrs[:, 0, :], d, OP.mult)
                    hn = hp.tile([128, BC], mybir.dt.bfloat16, tag=f"h{c}")
                    nc.vector.tensor_tensor(hn[0:U, :], h[0:U, :], e, OP.add)
                    nc.sync.dma_start(out=state[c, t, :, :], in_=hn[0:U, :])
                    hprev[c] = hn

            for c in range(NCHUNK):
                lt = singles.tile([128, BC], mybir.dt.bfloat16, tag=f"last{c}")
                nc.vector.tensor_copy(lt[0:U, :], hprev[c][0:U, :])
                last_tiles.append(lt)
                lo = singles.tile([U, BC], mybir.dt.float32, tag=f"lasto{c}")
                nc.vector.tensor_copy(lo, hprev[c][0:U, :])
                nc.sync.dma_start(out=lastout[c, :, :], in_=lo)

        # ---------------- attention ----------------
        with ExitStack() as ctx:
            sp = ctx.enter_context(tc.tile_pool(name="sp", bufs=4))
            gp2 = ctx.enter_context(tc.tile_pool(name="gp2", bufs=3))
            ps = ctx.enter_context(tc.tile_pool(name="ps", bufs=1, space="PSUM"))
            pa = ctx.enter_context(tc.tile_pool(name="pa", bufs=1, space="PSUM"))
            po = ctx.enter_context(tc.tile_pool(name="po", bufs=1, space="PSUM"))

            for c in range(NCHUNK):
                acc = po.tile([128, 512], mybir.dt.float32, tag=f"acc{c}")
                for t in range(T):
                    st = sp.tile([U, BC], mybir.dt.bfloat16, tag=f"st{c}")
                    nc.sync.dma_start(out=st, in_=state[c, t, :, :])
                    sb = ps.tile([128, 512], mybir.dt.float32, tag=f"sb{c}")
                    nc.tensor.matmul(sb[0:U, 0:BC], lhsT=A2b, rhs=st, start=True, stop=False)
                    nc.tensor.matmul(sb[0:U, 0:BC], lhsT=A1b, rhs=last_tiles[c][0:U, :], start=False, stop=True)
                    g = gp2.tile([U, BC], mybir.dt.bfloat16, tag=f"g{c}")
                    nc.scalar.activation(g, sb[0:U, 0:BC], AF.Sigmoid)
                    al = pa.tile([128, 512], mybir.dt.float32, tag=f"al{c}")
                    nc.tensor.matmul(al[0:U, 0:BC], lhsT=Vr, rhs=g, start=True, stop=True)
                    tmp = gp2.tile([U, BC], mybir.dt.bfloat16, tag=f"tmp{c}")
                    nc.vector.tensor_tensor(tmp, al[0:U, 0:BC], st, OP.mult)
                    nc.tensor.matmul(acc[0:U, 0:BC], lhsT=I100, rhs=tmp,
                                     start=(t == 0), stop=(t == T - 1))
                osb = gp2.tile([U, BC], mybir.dt.float32, tag=f"osb{c}")
                nc.vector.tensor_copy(osb, acc[0:U, 0:BC])
                nc.sync.dma_start(out=outraw[c, :, :], in_=osb)

    nc.compile()
    return nc


def _prep_weights(kernel_w, rec_kernel, bias):
    b0, b1 = bias[0], bias[1]
    w = {}
    KzN = np.zeros((128, U), np.float32)
    KzN[:E] = -kernel_w[:, :U]
    KzN[100, :] = -40.0
    KzN[101, :] = -(b0[:U] + b1[:U])
    Kr = np.zeros((128, U), np.float32)
    Kr[:E] = kernel_w[:, U:2 * U]
    Kr[101, :] = b0[U:2 * U] + b1[U:2 * U]
    Kh = np.zeros((128, U), np.float32)
    Kh[:E] = kernel_w[:, 2 * U:]
    Kh[101, :] = b0[2 * U:]
    w["wKzN"], w["wKr"], w["wKh"] = KzN, Kr, Kh
    w["wRzN"] = -rec_kernel[:, :U]
    w["wRr"] = rec_kernel[:, U:2 * U]
    w["wRh"] = rec_kernel[:, 2 * U:]
    w["wb1h"] = b1[2 * U:][None, :]
    return {k: v.astype(bf16) for k, v in w.items()}


def kernel(session_hidden, mask, kernel, rec_kernel, bias, A1_w, A2_w, v, _trace=False):
    session_hidden = np.asarray(session_hidden, np.float32)
    mask = np.asarray(mask, np.float32)
    kernel_w = np.asarray(kernel, np.float32)
    rec_kernel = np.asarray(rec_kernel, np.float32)
    bias = np.asarray(bias, np.float32)
    A1_w = np.asarray(A1_w, np.float32)
    A2_w = np.asarray(A2_w, np.float32)
    v = np.asarray(v, np.float32)

    if "nc" not in _CACHE:
        _CACHE["nc"] = _build()
    nc = _CACHE["nc"]

    w = _prep_weights(kernel_w, rec_kernel, bias)
    w["wA1"] = A1_w.astype(bf16)
    w["wA2"] = A2_w.astype(bf16)
    w["wVr"] = np.broadcast_to(v[0][:, None], (U, U)).astype(bf16).copy()
    w["wI"] = np.eye(U, dtype=np.float32).astype(bf16)

    # xaug: [T, NCHUNK, 128, BC] per core; rows 0:100 = x^T, 100 = 1-m, 101 = 1
    x = session_hidden.reshape(NCORES, NCHUNK, BC, T, E)
    m = mask.reshape(NCORES, NCHUNK, BC, T)
    in_maps = []
    for k in range(NCORES):
        xa = np.zeros((T, NCHUNK, 128, BC), np.float32)
        xa[:, :, :E, :] = x[k].transpose(2, 0, 3, 1)   # [T, chunk, E, BC]
        xa[:, :, 100, :] = 1.0 - m[k].transpose(2, 0, 1)
        xa[:, :, 101, :] = 1.0
        im = dict(w)
        im["xaug"] = xa.astype(bf16)
        in_maps.append(im)

    res = bass_utils.run_bass_kernel_spmd(nc, in_maps, core_ids=list(range(NCORES)),
                                          trace=_trace)
    _CACHE["last_res"] = res

    out_raw = np.zeros((B, U), np.float32)
    last = np.zeros((B, U), np.float32)
    for k in range(NCORES):
        r = res.results[k]
        for c in range(NCHUNK):
            sl = slice(k * PERCORE + c * BC, k * PERCORE + (c + 1) * BC)
            out_raw[sl] = np.asarray(r["outraw"][c]).T.astype(np.float32)
            last[sl] = np.asarray(r["lastout"][c]).T.astype(np.float32)

    # host correction for masked timesteps (device used last@A1 term for ALL t)
    lengths = mask.sum(1)
    sl_ = last @ A2_w
    c_ = last @ A1_w
    sig = lambda a: 1.0 / (1.0 + np.exp(-a))
    a_corr = (sig(sl_ + c_) - sig(sl_)) @ v[0]
    out = out_raw - (T - lengths)[:, None] * a_corr[:, None] * last
    return out.astype(np.float32)

